# revision 11
# baseline (speedup 1.0000x reference)
"""Trainium2 Bass kernel for a stride-2 4x4 ConvTranspose2d with
per-kernel-position bias (bias added before the overlap-add fold).

Shapes (hardcoded):
  x:      (8, 256, 64, 64)  f32
  weight: (128, 256, 4, 4)  f32
  bias:   (128, 4, 4)       f32
  out:    (8, 128, 130, 130) f32   [nh = (64-1)*2+4 = 130]

Strategy: data-parallel over batch — one sample per NeuronCore, 8 cores.
Per core the deconv is computed as 4 output-phase planes (p%2, q%2), each
a 65x65 image. Each phase plane is a sum of 4 shifted matmuls
(kernel positions sharing that parity) accumulated directly in PSUM:

  plane[o, P, Q] = sum_{a,b in {0,1}} W[:, :, py+2a, px+2b]^T @ xp[:, P-a+1, Q-b+1]

where xp is x zero-padded to 66x66. Matmuls run as fp32r (full-rate fp32
on the PE array). The per-position bias is folded in as a per-partition
scalar during the PSUM->SBUF drain, with small edge/corner corrections
(boundary pixels receive fewer kernel-position contributions).
"""

import numpy as np

B, CI, H, W = 8, 256, 64, 64
CO, KH, KW = 128, 4, 4
NH = NW = 130
NP = 65          # phase plane side
PADH = 66        # padded x rows
PADW = 68        # padded x cols (fp32r needs even innermost matmul counts;
                 # we compute 66-wide rows and drain the valid 65)
NQ = 66          # matmul output row width (even), col 65 is garbage
NCORES = 8

# P-row chunking of a phase plane: free-dim per chunk must fit one PSUM
# bank (<=512 f32) and stay >=256 for full-rate fp32r.
CHUNK_ROWS = [7, 7, 7, 7, 7, 6, 6, 6, 6, 6]
assert sum(CHUNK_ROWS) == NP


def _build_nc():
    import concourse.mybir as mybir
    import concourse.tile as tile
    from concourse import bacc

    f32 = mybir.dt.float32
    f32r = mybir.dt.float32r

    # Bacc (not plain Bass): its compile() runs generate_event_semaphores,
    # legalizing Tile's multi-wait instructions to the 1-wait-per-inst
    # hardware constraint the walrus in this container enforces.
    nc = bacc.Bacc(None)
    xp_ext = nc.declare_dram_parameter("xp", [2, 128, PADH * PADW], f32r, isOutput=False)
    wt_ext = nc.declare_dram_parameter("wt", [2, 128, 16 * CO], f32r, isOutput=False)
    bv_ext = nc.declare_dram_parameter("bv", [128, 4 * 9], f32, isOutput=False)
    out_ext = nc.declare_dram_parameter("out", [CO, NH, NW], f32, isOutput=True)

    N_XGRP = 4
    XROW_GRPS = [(0, 17), (17, 34), (34, 51), (51, 66)]

    with tile.TileContext(nc) as tc:
        with (
            tc.tile_pool(name="const", bufs=1) as cpool,
            tc.tile_pool(name="psum", bufs=4, space="PSUM") as ppool,
        ):
            w_t = cpool.tile([128, 2, 16, CO], f32r, tag="w")
            xp_t = cpool.tile([128, 2, PADH, PADW], f32r, tag="xp")
            bv_t = cpool.tile([128, 4, 9], f32, tag="bv")
            out_ev = cpool.tile([128, NP, NW], f32, tag="oev")
            out_od = cpool.tile([128, NP, NW], f32, tag="ood")

            nc.sync.dma_start(bv_t[:], bv_ext[:])
            for kt in range(2):
                nc.sync.dma_start(w_t[:, kt], wt_ext[kt])
            for kt in range(2):
                for (r0, r1) in XROW_GRPS:
                    nc.sync.dma_start(
                        xp_t[:, kt, r0:r1, :], xp_ext[kt, :, r0 * PADW:r1 * PADW]
                    )

            for (py, px) in ((0, 0), (0, 1), (1, 0), (1, 1)):
                out_t = out_ev if py == 0 else out_od
                pidx = py * 2 + px
                pstart = 0
                for pn in CHUNK_ROWS:
                    ps = ppool.tile([128, 7, NQ], f32, tag="acc")
                    mm = 0
                    for a in (0, 1):
                        for b in (0, 1):
                            kpos = (py + 2 * a) * 4 + (px + 2 * b)
                            for kt in (0, 1):
                                lhsT = w_t[:, kt, kpos, :]
                                rhs = xp_t[
                                    :, kt,
                                    pstart + 1 - a: pstart + 1 - a + pn,
                                    1 - b: 1 - b + NQ,
                                ]
                                nc.tensor.matmul(
                                    ps[:, :pn, :], lhsT, rhs,
                                    start=(mm == 0), stop=(mm == 7),
                                )
                                mm += 1
                    # drain + interior bias (col 65 of ps is garbage, skip it)
                    nc.vector.tensor_scalar_add(
                        out_t[:, pstart:pstart + pn, px::2],
                        ps[:, :pn, 0:NP],
                        bv_t[:, pidx, 0:1],
                    )
                    pstart += pn
                # edge corrections (boundary pixels get fewer kpos contributions)
                for (sl, k) in (
                    (out_t[:, 0:1, px::2], 1),       # P=0 row
                    (out_t[:, 64:65, px::2], 2),     # P=64 row
                    (out_t[:, :, px:px + 1], 3),     # Q=0 col
                    (out_t[:, :, px + 128:px + 129], 4),  # Q=64 col
                    (out_t[:, 0:1, px:px + 1], 5),
                    (out_t[:, 0:1, px + 128:px + 129], 6),
                    (out_t[:, 64:65, px:px + 1], 7),
                    (out_t[:, 64:65, px + 128:px + 129], 8),
                ):
                    nc.vector.tensor_scalar_add(sl, sl, bv_t[:, pidx, k:k + 1])
                if (py, px) == (0, 1):
                    nc.sync.dma_start(out_ext[:, 0::2, :], out_ev[:])
                elif (py, px) == (1, 1):
                    nc.sync.dma_start(out_ext[:, 1::2, :], out_od[:])
    nc.compile()
    return nc


def _host_prep(x, weight, bias):
    # padded, i-tiled x: [B, 2, 128, 66, 68]
    xp = np.zeros((B, 2, 128, PADH, PADW), dtype=np.float32)
    xp[:, :, :, 1:65, 1:65] = x.reshape(B, 2, 128, H, W)
    xp = np.ascontiguousarray(xp.reshape(B, 2, 128, PADH * PADW))

    # weights as lhsT: wt[kt, i, kpos, o] = weight[o, kt*128+i, ky, kx]
    wr = weight.reshape(CO, 2, 128, 16)
    wt = np.ascontiguousarray(wr.transpose(1, 2, 3, 0)).reshape(2, 128, 16 * CO)

    # bias vectors [128, 4, 9]: interior sum + 4 edge + 4 corner corrections
    bv = np.zeros((128, 4, 9), dtype=np.float32)
    bias = bias.astype(np.float32)
    for py in range(2):
        for px in range(2):
            p = py * 2 + px
            b00 = bias[:, py, px]
            b01 = bias[:, py, px + 2]
            b10 = bias[:, py + 2, px]
            b11 = bias[:, py + 2, px + 2]
            bv[:, p, 0] = b00 + b01 + b10 + b11
            bv[:, p, 1] = -(b10 + b11)   # P=0 row (a=1 invalid)
            bv[:, p, 2] = -(b00 + b01)   # P=64 row (a=0 invalid)
            bv[:, p, 3] = -(b01 + b11)   # Q=0 col (b=1 invalid)
            bv[:, p, 4] = -(b00 + b10)   # Q=64 col (b=0 invalid)
            bv[:, p, 5] = b11            # corner (0,0)
            bv[:, p, 6] = b10            # corner (0,64)
            bv[:, p, 7] = b01            # corner (64,0)
            bv[:, p, 8] = b00            # corner (64,64)
    bv = bv.reshape(128, 36)
    return xp, wt, bv


_NC_CACHE = {}


def _get_nc():
    if "nc" not in _NC_CACHE:
        _NC_CACHE["nc"] = _build_nc()
    return _NC_CACHE["nc"]


def kernel(x, weight, bias, _trace=False, _trace_kwargs=None):
    from concourse.bass_utils import run_bass_kernel_spmd

    x = np.asarray(x, dtype=np.float32)
    weight = np.asarray(weight, dtype=np.float32)
    bias = np.asarray(bias, dtype=np.float32)
    xp, wt, bv = _host_prep(x, weight, bias)

    nc = _get_nc()
    in_maps = [{"xp": xp[b], "wt": wt, "bv": bv} for b in range(B)]
    res = run_bass_kernel_spmd(
        nc, in_maps, list(range(NCORES)),
        trace=_trace, **(_trace_kwargs or {}),
    )
    out = np.stack([res.results[b]["out"] for b in range(B)])
    if _trace:
        kernel._last_results = res
    return out


# revision 13
# speedup vs baseline: 1.2270x; 1.2270x over previous
"""Trainium2 Bass kernel for a stride-2 4x4 ConvTranspose2d with
per-kernel-position bias (bias added before the overlap-add fold).

Shapes (hardcoded):
  x:      (8, 256, 64, 64)  f32
  weight: (128, 256, 4, 4)  f32
  bias:   (128, 4, 4)       f32
  out:    (8, 128, 130, 130) f32   [nh = (64-1)*2+4 = 130]

Strategy: data-parallel over batch — one sample per NeuronCore, 8 cores.
Per core the deconv is computed as 4 output-phase planes (p%2, q%2), each
a 65x65 image. Each phase plane is the sum of 4 shifted matmuls (the
kernel positions sharing that parity) accumulated directly in PSUM:

  plane[o, P, Q] = sum_{a,b in {0,1}} W[:, :, py+2a, px+2b]^T @ xp[:, P-a+1, Q-b+1]

with xp zero-padded so out-of-range taps contribute zero. Matmuls run as
fp32r (full-rate fp32 on the PE array; requires even innermost counts,
hence the 66-wide compute rows of which 65 are kept). The bias is folded
in as a per-partition scalar during the PSUM->SBUF drain, with small
edge/corner corrections for boundary pixels that receive fewer
kernel-position contributions. Phase planes are stored contiguously and
DMA'd out in halves as soon as their rows are final; the host interleaves
the 4 planes into the strided (130,130) output.
"""

import numpy as np

B, CI, H, W = 8, 256, 64, 64
CO, KH, KW = 128, 4, 4
NH = NW = 130
NP = 65          # phase plane side
PADH = 66        # padded x rows
PADW = 68        # padded x cols (fp32r needs even innermost matmul counts;
                 # we compute 66-wide rows and drain the valid 65)
NQ = 66          # matmul output row width (even); col 65 is garbage
NCORES = 8

# P-row chunking of a phase plane: free-dim per chunk must fit one PSUM
# bank (<=512 f32) and stay >=256 for full-rate fp32r.
CHUNK_ROWS = [7, 7, 7, 7, 7, 6, 6, 6, 6, 6]
assert sum(CHUNK_ROWS) == NP
HALF_ROWS = 35   # rows 0..34 complete after chunk 4; 35..64 after chunk 9

XROW_GRPS = [(0, 22), (22, 44), (44, 66)]


def _build_nc():
    import concourse.mybir as mybir
    import concourse.tile as tile
    from concourse import bacc

    f32 = mybir.dt.float32
    f32r = mybir.dt.float32r

    # Bacc (not plain Bass): its compile() runs generate_event_semaphores,
    # legalizing Tile's multi-wait instructions to the 1-wait-per-inst
    # hardware constraint the walrus in this container enforces.
    nc = bacc.Bacc(None)
    xp_ext = nc.declare_dram_parameter("xp", [2, 128, PADH * PADW], f32r, isOutput=False)
    wt_ext = nc.declare_dram_parameter("wt", [2, 128, 16 * CO], f32r, isOutput=False)
    bv_ext = nc.declare_dram_parameter("bv", [128, 4 * 9], f32, isOutput=False)
    out_ext = nc.declare_dram_parameter("out", [4, CO, NP * NP], f32, isOutput=True)

    with tile.TileContext(nc) as tc:
        with (
            tc.tile_pool(name="const", bufs=1) as cpool,
            tc.tile_pool(name="psum", bufs=4, space="PSUM") as ppool,
        ):
            w_t = cpool.tile([128, 2, 16, CO], f32r, tag="w")
            xp_t = cpool.tile([128, 2, PADH, PADW], f32r, tag="xp")
            bv_t = cpool.tile([128, 4, 9], f32, tag="bv")
            planes = [
                cpool.tile([128, NP, NP], f32, tag=f"plane{p}", name=f"plane{p}")
                for p in range(4)
            ]

            # Inputs: weights/bias on the ACT HWDGE ring, x on gpsimd SWDGE
            # queues — keeps the SP ring free for output and the trigger
            # streams parallel. First-needed row groups go first.
            nc.scalar.dma_start(bv_t[:], bv_ext[:])
            for kt in range(2):
                nc.scalar.dma_start(w_t[:, kt], wt_ext[kt])
            for (r0, r1) in XROW_GRPS:
                for kt in range(2):
                    nc.gpsimd.dma_start(
                        xp_t[:, kt, r0:r1, :], xp_ext[kt, :, r0 * PADW:r1 * PADW]
                    )

            for (py, px) in ((0, 0), (0, 1), (1, 0), (1, 1)):
                pidx = py * 2 + px
                plane = planes[pidx]
                pstart = 0
                for ci, pn in enumerate(CHUNK_ROWS):
                    ps = ppool.tile([128, 7, NQ], f32, tag="acc")
                    mm = 0
                    for a in (0, 1):
                        for b in (0, 1):
                            kpos = (py + 2 * a) * 4 + (px + 2 * b)
                            for kt in (0, 1):
                                lhsT = w_t[:, kt, kpos, :]
                                rhs = xp_t[
                                    :, kt,
                                    pstart + 1 - a: pstart + 1 - a + pn,
                                    1 - b: 1 - b + NQ,
                                ]
                                nc.tensor.matmul(
                                    ps[:, :pn, :], lhsT, rhs,
                                    start=(mm == 0), stop=(mm == 7),
                                )
                                mm += 1
                    # drain + interior bias (col 65 of ps is garbage, skip it)
                    nc.vector.tensor_scalar_add(
                        plane[:, pstart:pstart + pn, :],
                        ps[:, :pn, 0:NP],
                        bv_t[:, pidx, 0:1],
                    )
                    pstart += pn

                    if ci == 4:
                        # rows 0..34 final modulo their edge corrections
                        for (sl, k) in (
                            (plane[:, 0:1, :], 1),                    # P=0 row
                            (plane[:, 0:HALF_ROWS, 0:1], 3),          # Q=0 col (top)
                            (plane[:, 0:HALF_ROWS, 64:65], 4),        # Q=64 col (top)
                            (plane[:, 0:1, 0:1], 5),                  # corner (0,0)
                            (plane[:, 0:1, 64:65], 6),                # corner (0,64)
                        ):
                            nc.vector.tensor_scalar_add(sl, sl, bv_t[:, pidx, k:k + 1])
                        nc.sync.dma_start(
                            out_ext[pidx, :, 0:HALF_ROWS * NP],
                            plane[:, 0:HALF_ROWS, :],
                        )
                    elif ci == 9:
                        for (sl, k) in (
                            (plane[:, 64:65, :], 2),                  # P=64 row
                            (plane[:, HALF_ROWS:NP, 0:1], 3),         # Q=0 col (bot)
                            (plane[:, HALF_ROWS:NP, 64:65], 4),       # Q=64 col (bot)
                            (plane[:, 64:65, 0:1], 7),                # corner (64,0)
                            (plane[:, 64:65, 64:65], 8),              # corner (64,64)
                        ):
                            nc.vector.tensor_scalar_add(sl, sl, bv_t[:, pidx, k:k + 1])
                        nc.sync.dma_start(
                            out_ext[pidx, :, HALF_ROWS * NP:],
                            plane[:, HALF_ROWS:NP, :],
                        )
    nc.compile()
    return nc


def _host_prep(x, weight, bias):
    # padded, i-tiled x: [B, 2, 128, 66, 68]
    xp = np.zeros((B, 2, 128, PADH, PADW), dtype=np.float32)
    xp[:, :, :, 1:65, 1:65] = x.reshape(B, 2, 128, H, W)
    xp = np.ascontiguousarray(xp.reshape(B, 2, 128, PADH * PADW))

    # weights as lhsT: wt[kt, i, kpos, o] = weight[o, kt*128+i, ky, kx]
    wr = weight.reshape(CO, 2, 128, 16)
    wt = np.ascontiguousarray(wr.transpose(1, 2, 3, 0)).reshape(2, 128, 16 * CO)

    # bias vectors [128, 4, 9]: interior sum + 4 edge + 4 corner corrections
    bv = np.zeros((128, 4, 9), dtype=np.float32)
    bias = bias.astype(np.float32)
    for py in range(2):
        for px in range(2):
            p = py * 2 + px
            b00 = bias[:, py, px]
            b01 = bias[:, py, px + 2]
            b10 = bias[:, py + 2, px]
            b11 = bias[:, py + 2, px + 2]
            bv[:, p, 0] = b00 + b01 + b10 + b11
            bv[:, p, 1] = -(b10 + b11)   # P=0 row (a=1 invalid)
            bv[:, p, 2] = -(b00 + b01)   # P=64 row (a=0 invalid)
            bv[:, p, 3] = -(b01 + b11)   # Q=0 col (b=1 invalid)
            bv[:, p, 4] = -(b00 + b10)   # Q=64 col (b=0 invalid)
            bv[:, p, 5] = b11            # corner (0,0)
            bv[:, p, 6] = b10            # corner (0,64)
            bv[:, p, 7] = b01            # corner (64,0)
            bv[:, p, 8] = b00            # corner (64,64)
    bv = bv.reshape(128, 36)
    return xp, wt, bv


_NC_CACHE = {}


def _get_nc():
    if "nc" not in _NC_CACHE:
        _NC_CACHE["nc"] = _build_nc()
    return _NC_CACHE["nc"]


def kernel(x, weight, bias, _trace=False, _trace_kwargs=None):
    from concourse.bass_utils import run_bass_kernel_spmd

    x = np.asarray(x, dtype=np.float32)
    weight = np.asarray(weight, dtype=np.float32)
    bias = np.asarray(bias, dtype=np.float32)
    xp, wt, bv = _host_prep(x, weight, bias)

    nc = _get_nc()
    in_maps = [{"xp": xp[b], "wt": wt, "bv": bv} for b in range(B)]
    res = run_bass_kernel_spmd(
        nc, in_maps, list(range(NCORES)),
        trace=_trace, **(_trace_kwargs or {}),
    )
    out = np.empty((B, CO, NH, NW), dtype=np.float32)
    for b in range(B):
        ph = res.results[b]["out"].reshape(4, CO, NP, NP)
        for py in range(2):
            for px in range(2):
                out[b, :, py::2, px::2] = ph[py * 2 + px]
    if _trace:
        kernel._last_results = res
    return out


# revision 17
# speedup vs baseline: 1.3255x; 1.0803x over previous
"""Trainium2 Bass kernel for a stride-2 4x4 ConvTranspose2d with
per-kernel-position bias (bias added before the overlap-add fold).

Shapes (hardcoded):
  x:      (8, 256, 64, 64)  f32
  weight: (128, 256, 4, 4)  f32
  bias:   (128, 4, 4)       f32
  out:    (8, 128, 130, 130) f32   [nh = (64-1)*2+4 = 130]

Strategy: data-parallel over batch — one sample per NeuronCore, 8 cores.
Per core the deconv is computed as 4 output-phase planes (p%2, q%2), each
a 65x65 image. Each phase plane is the sum of 4 shifted matmuls (the
kernel positions sharing that parity) accumulated directly in PSUM:

  plane[o, P, Q] = sum_{a,b in {0,1}} W[:, :, py+2a, px+2b]^T @ xp[:, P-a+1, Q-b+1]

with xp zero-padded so out-of-range taps contribute zero. Matmuls run as
fp32r (full-rate fp32 on the PE array; requires even innermost counts,
hence the 66-wide compute rows of which 65 are kept). The bias is folded
in as a per-partition scalar during the PSUM->SBUF drain, with small
edge/corner corrections for boundary pixels that receive fewer
kernel-position contributions. Phase planes are stored contiguously and
DMA'd out in halves as soon as their rows are final; the host interleaves
the 4 planes into the strided (130,130) output.
"""

import numpy as np

B, CI, H, W = 8, 256, 64, 64
CO, KH, KW = 128, 4, 4
NH = NW = 130
NP = 65          # phase plane side
PADH = 66        # padded x rows
PADW = 68        # padded x cols (fp32r needs even innermost matmul counts;
                 # we compute 66-wide rows and drain the valid 65)
NQ = 66          # matmul output row width (even); col 65 is garbage
NCORES = 8

# P-row chunking of a phase plane: free-dim per chunk must fit one PSUM
# bank (<=512 f32) and stay >=256 for full-rate fp32r.
CHUNK_ROWS = [7, 7, 7, 7, 7, 6, 6, 6, 6, 6]
assert sum(CHUNK_ROWS) == NP
HALF_ROWS = 35   # rows 0..34 complete after chunk 4; 35..64 after chunk 9

XROW_GRPS = [(0, 11), (11, 22), (22, 44), (44, 66)]


def _build_nc():
    import concourse.mybir as mybir
    import concourse.tile as tile
    from concourse import bacc

    f32 = mybir.dt.float32
    f32r = mybir.dt.float32r

    # Bacc (not plain Bass): its compile() runs generate_event_semaphores,
    # legalizing Tile's multi-wait instructions to the 1-wait-per-inst
    # hardware constraint the walrus in this container enforces.
    nc = bacc.Bacc(None)
    xp_ext = nc.declare_dram_parameter("xp", [2, 128, PADH * PADW], f32r, isOutput=False)
    wt_ext = nc.declare_dram_parameter("wt", [2, 128, 16 * CO], f32r, isOutput=False)
    bv_ext = nc.declare_dram_parameter("bv", [128, 4 * 9], f32, isOutput=False)
    out_ext = nc.declare_dram_parameter("out", [4, CO, NP * NP], f32, isOutput=True)

    with tile.TileContext(nc) as tc:
        with (
            tc.tile_pool(name="const", bufs=1) as cpool,
            tc.tile_pool(name="psum", bufs=4, space="PSUM") as ppool,
        ):
            # w_t layout: [i, kt, phase, j=(a,b), o] — phase-blocked so each
            # phase's weights arrive in one small early DMA.
            w_t = cpool.tile([128, 2, 4, 4, CO], f32r, tag="w")
            xp_t = cpool.tile([128, 2, PADH, PADW], f32r, tag="xp")
            bv_t = cpool.tile([128, 4, 9], f32, tag="bv")
            planes = [
                cpool.tile([128, NP, NP], f32, tag=f"plane{p}", name=f"plane{p}")
                for p in range(4)
            ]

            # Inputs split across the two HWDGE rings (ACT: weights/bias,
            # SP: x), first-needed pieces first, so the first matmul is
            # gated by ~1.3MB not the full 6.6MB input load.
            nc.scalar.dma_start(bv_t[:], bv_ext[:])
            for ph in range(4):
                for kt in range(2):
                    nc.scalar.dma_start(
                        w_t[:, kt, ph], wt_ext[kt, :, ph * 4 * CO:(ph + 1) * 4 * CO]
                    )
            for (r0, r1) in XROW_GRPS:
                for kt in range(2):
                    nc.sync.dma_start(
                        xp_t[:, kt, r0:r1, :], xp_ext[kt, :, r0 * PADW:r1 * PADW]
                    )

            for (py, px) in ((0, 0), (0, 1), (1, 0), (1, 1)):
                pidx = py * 2 + px
                plane = planes[pidx]
                pstart = 0
                for ci, pn in enumerate(CHUNK_ROWS):
                    ps = ppool.tile([128, 7, NQ], f32, tag="acc")
                    mm = 0
                    for a in (0, 1):
                        for b in (0, 1):
                            for kt in (0, 1):
                                lhsT = w_t[:, kt, pidx, a * 2 + b, :]
                                rhs = xp_t[
                                    :, kt,
                                    pstart + 1 - a: pstart + 1 - a + pn,
                                    1 - b: 1 - b + NQ,
                                ]
                                nc.tensor.matmul(
                                    ps[:, :pn, :], lhsT, rhs,
                                    start=(mm == 0), stop=(mm == 7),
                                )
                                mm += 1
                    # drain + interior bias (col 65 of ps is garbage, skip it)
                    nc.vector.tensor_scalar_add(
                        plane[:, pstart:pstart + pn, :],
                        ps[:, :pn, 0:NP],
                        bv_t[:, pidx, 0:1],
                    )
                    pstart += pn

                    if ci == 4:
                        # rows 0..34 final modulo their edge corrections
                        for (sl, k) in (
                            (plane[:, 0:1, :], 1),                    # P=0 row
                            (plane[:, 0:HALF_ROWS, 0:1], 3),          # Q=0 col (top)
                            (plane[:, 0:HALF_ROWS, 64:65], 4),        # Q=64 col (top)
                            (plane[:, 0:1, 0:1], 5),                  # corner (0,0)
                            (plane[:, 0:1, 64:65], 6),                # corner (0,64)
                        ):
                            nc.vector.tensor_scalar_add(sl, sl, bv_t[:, pidx, k:k + 1])
                        nc.sync.dma_start(
                            out_ext[pidx, :, 0:HALF_ROWS * NP],
                            plane[:, 0:HALF_ROWS, :],
                        )
                    elif ci == 9:
                        for (sl, k) in (
                            (plane[:, 64:65, :], 2),                  # P=64 row
                            (plane[:, HALF_ROWS:NP, 0:1], 3),         # Q=0 col (bot)
                            (plane[:, HALF_ROWS:NP, 64:65], 4),       # Q=64 col (bot)
                            (plane[:, 64:65, 0:1], 7),                # corner (64,0)
                            (plane[:, 64:65, 64:65], 8),              # corner (64,64)
                        ):
                            nc.vector.tensor_scalar_add(sl, sl, bv_t[:, pidx, k:k + 1])
                        nc.sync.dma_start(
                            out_ext[pidx, :, HALF_ROWS * NP:],
                            plane[:, HALF_ROWS:NP, :],
                        )
    nc.compile()
    return nc


def _host_prep(x, weight, bias):
    # padded, i-tiled x: [B, 2, 128, 66, 68]
    xp = np.zeros((B, 2, 128, PADH, PADW), dtype=np.float32)
    xp[:, :, :, 1:65, 1:65] = x.reshape(B, 2, 128, H, W)
    xp = np.ascontiguousarray(xp.reshape(B, 2, 128, PADH * PADW))

    # weights as lhsT, phase-blocked:
    # wt[kt, i, phase, j=(a,b), o] = weight[o, kt*128+i, py+2a, px+2b]
    wr = weight.reshape(CO, 2, 128, 4, 4)
    wt = np.empty((2, 128, 4, 4, CO), dtype=np.float32)
    for py in range(2):
        for px in range(2):
            for a in range(2):
                for bb in range(2):
                    wt[:, :, py * 2 + px, a * 2 + bb, :] = (
                        wr[:, :, :, py + 2 * a, px + 2 * bb].transpose(1, 2, 0)
                    )
    wt = np.ascontiguousarray(wt).reshape(2, 128, 16 * CO)

    # bias vectors [128, 4, 9]: interior sum + 4 edge + 4 corner corrections
    bv = np.zeros((128, 4, 9), dtype=np.float32)
    bias = bias.astype(np.float32)
    for py in range(2):
        for px in range(2):
            p = py * 2 + px
            b00 = bias[:, py, px]
            b01 = bias[:, py, px + 2]
            b10 = bias[:, py + 2, px]
            b11 = bias[:, py + 2, px + 2]
            bv[:, p, 0] = b00 + b01 + b10 + b11
            bv[:, p, 1] = -(b10 + b11)   # P=0 row (a=1 invalid)
            bv[:, p, 2] = -(b00 + b01)   # P=64 row (a=0 invalid)
            bv[:, p, 3] = -(b01 + b11)   # Q=0 col (b=1 invalid)
            bv[:, p, 4] = -(b00 + b10)   # Q=64 col (b=0 invalid)
            bv[:, p, 5] = b11            # corner (0,0)
            bv[:, p, 6] = b10            # corner (0,64)
            bv[:, p, 7] = b01            # corner (64,0)
            bv[:, p, 8] = b00            # corner (64,64)
    bv = bv.reshape(128, 36)
    return xp, wt, bv


_NC_CACHE = {}


def _get_nc():
    if "nc" not in _NC_CACHE:
        _NC_CACHE["nc"] = _build_nc()
    return _NC_CACHE["nc"]


def kernel(x, weight, bias, _trace=False, _trace_kwargs=None):
    from concourse.bass_utils import run_bass_kernel_spmd

    x = np.asarray(x, dtype=np.float32)
    weight = np.asarray(weight, dtype=np.float32)
    bias = np.asarray(bias, dtype=np.float32)
    xp, wt, bv = _host_prep(x, weight, bias)

    nc = _get_nc()
    in_maps = [{"xp": xp[b], "wt": wt, "bv": bv} for b in range(B)]
    res = run_bass_kernel_spmd(
        nc, in_maps, list(range(NCORES)),
        trace=_trace, **(_trace_kwargs or {}),
    )
    out = np.empty((B, CO, NH, NW), dtype=np.float32)
    for b in range(B):
        ph = res.results[b]["out"].reshape(4, CO, NP, NP)
        for py in range(2):
            for px in range(2):
                out[b, :, py::2, px::2] = ph[py * 2 + px]
    if _trace:
        kernel._last_results = res
    return out


# revision 22
# speedup vs baseline: 1.3353x; 1.0074x over previous
"""Trainium2 Bass kernel for a stride-2 4x4 ConvTranspose2d with
per-kernel-position bias (bias added before the overlap-add fold).

Shapes (hardcoded):
  x:      (8, 256, 64, 64)  f32
  weight: (128, 256, 4, 4)  f32
  bias:   (128, 4, 4)       f32
  out:    (8, 128, 130, 130) f32   [nh = (64-1)*2+4 = 130]

Strategy: data-parallel over batch — one sample per NeuronCore, 8 cores.
Per core the deconv is computed as 4 output-phase planes (p%2, q%2), each
a 65x65 image. Each phase plane is the sum of 4 shifted matmuls (the
kernel positions sharing that parity) accumulated directly in PSUM:

  plane[o, P, Q] = sum_{a,b in {0,1}} W[:, :, py+2a, px+2b]^T @ xp[:, P-a+1, Q-b+1]

with xp zero-padded so out-of-range taps contribute zero. Matmuls run as
fp32r (full-rate fp32 on the PE array; requires even innermost counts,
hence the 66-wide compute rows of which 65 are kept). The bias is folded
in as a per-partition scalar during the PSUM->SBUF drain, with small
edge/corner corrections for boundary pixels that receive fewer
kernel-position contributions. Phase planes are stored contiguously and
DMA'd out in halves as soon as their rows are final; the host interleaves
the 4 planes into the strided (130,130) output.
"""

import numpy as np

B, CI, H, W = 8, 256, 64, 64
CO, KH, KW = 128, 4, 4
NH = NW = 130
NP = 65          # phase plane side
PADH = 66        # padded x rows
PADW = 68        # padded x cols (fp32r needs even innermost matmul counts;
                 # we compute 66-wide rows and drain the valid 65)
NQ = 66          # matmul output row width (even); col 65 is garbage
NCORES = 8

# P-row chunking of a phase plane: free-dim per chunk must fit one PSUM
# bank (<=512 f32) and stay >=256 for full-rate fp32r.
CHUNK_ROWS = [7, 7, 7, 7, 7, 6, 6, 6, 6, 6]
assert sum(CHUNK_ROWS) == NP
# after chunk index ci, rows [r0, r1) of the plane are complete
OUT_SPLITS = {4: (0, 35), 7: (35, 53), 9: (53, 65)}

XROW_GRPS = [(0, 11), (11, 22), (22, 44), (44, 66)]


def _build_nc():
    import concourse.mybir as mybir
    import concourse.tile as tile
    from concourse import bacc

    f32 = mybir.dt.float32
    f32r = mybir.dt.float32r

    # Bacc (not plain Bass): its compile() runs generate_event_semaphores,
    # legalizing Tile's multi-wait instructions to the 1-wait-per-inst
    # hardware constraint the walrus in this container enforces.
    nc = bacc.Bacc(None)
    xp_ext = nc.declare_dram_parameter("xp", [128, 2, PADH * PADW], f32r, isOutput=False)
    wt_ext = nc.declare_dram_parameter("wt", [128, 2, 16 * CO], f32r, isOutput=False)
    bv_ext = nc.declare_dram_parameter("bv", [128, 4 * 9], f32, isOutput=False)
    out_ext = nc.declare_dram_parameter("out", [4, CO, NP * NP], f32, isOutput=True)

    with tile.TileContext(nc) as tc:
        with (
            tc.tile_pool(name="const", bufs=1) as cpool,
            tc.tile_pool(name="psum", bufs=4, space="PSUM") as ppool,
        ):
            # w_t layout: [i, kt, phase, j=(a,b), o] — phase-blocked so each
            # phase's weights arrive in one small early DMA.
            w_t = cpool.tile([128, 2, 4, 4, CO], f32r, tag="w")
            xp_t = cpool.tile([128, 2, PADH, PADW], f32r, tag="xp")
            bv_t = cpool.tile([128, 4, 9], f32, tag="bv")
            planes = [
                cpool.tile([128, NP, NP], f32, tag=f"plane{p}", name=f"plane{p}")
                for p in range(4)
            ]

            # Inputs split across the two HWDGE rings, first-needed pieces
            # first, so the first matmul is gated by ~1.3MB not the full
            # 6.6MB input load. Late-needed weight phases ride behind x on
            # the SP ring.
            nc.scalar.dma_start(bv_t[:], bv_ext[:])
            for ph in (0, 1):
                nc.scalar.dma_start(
                    w_t[:, :, ph], wt_ext[:, :, ph * 4 * CO:(ph + 1) * 4 * CO]
                )
            for (r0, r1) in XROW_GRPS:
                nc.sync.dma_start(
                    xp_t[:, :, r0:r1, :], xp_ext[:, :, r0 * PADW:r1 * PADW]
                )
            for ph in (2, 3):
                nc.sync.dma_start(
                    w_t[:, :, ph], wt_ext[:, :, ph * 4 * CO:(ph + 1) * 4 * CO]
                )

            for (py, px) in ((0, 0), (0, 1), (1, 0), (1, 1)):
                pidx = py * 2 + px
                plane = planes[pidx]
                pstart = 0
                for ci, pn in enumerate(CHUNK_ROWS):
                    ps = ppool.tile([128, 7, NQ], f32, tag="acc")
                    mm = 0
                    for a in (0, 1):
                        for b in (0, 1):
                            for kt in (0, 1):
                                lhsT = w_t[:, kt, pidx, a * 2 + b, :]
                                rhs = xp_t[
                                    :, kt,
                                    pstart + 1 - a: pstart + 1 - a + pn,
                                    1 - b: 1 - b + NQ,
                                ]
                                nc.tensor.matmul(
                                    ps[:, :pn, :], lhsT, rhs,
                                    start=(mm == 0), stop=(mm == 7),
                                )
                                mm += 1
                    # drain + interior bias (col 65 of ps is garbage, skip it)
                    nc.vector.tensor_scalar_add(
                        plane[:, pstart:pstart + pn, :],
                        ps[:, :pn, 0:NP],
                        bv_t[:, pidx, 0:1],
                    )
                    pstart += pn

                    if ci in OUT_SPLITS:
                        # rows [r0, r1) are final once their edge corrections
                        # land; stream them out immediately.
                        r0, r1 = OUT_SPLITS[ci]
                        corr = [
                            (plane[:, r0:r1, 0:1], 3),      # Q=0 col
                            (plane[:, r0:r1, 64:65], 4),    # Q=64 col
                        ]
                        if ci == 4:
                            corr = [
                                (plane[:, 0:1, :], 1),      # P=0 row
                                (plane[:, 0:1, 0:1], 5),    # corner (0,0)
                                (plane[:, 0:1, 64:65], 6),  # corner (0,64)
                            ] + corr
                        elif ci == 9:
                            corr = [
                                (plane[:, 64:65, :], 2),      # P=64 row
                                (plane[:, 64:65, 0:1], 7),    # corner (64,0)
                                (plane[:, 64:65, 64:65], 8),  # corner (64,64)
                            ] + corr
                        for (sl, k) in corr:
                            nc.vector.tensor_scalar_add(sl, sl, bv_t[:, pidx, k:k + 1])
                        nc.sync.dma_start(
                            out_ext[pidx, :, r0 * NP:r1 * NP],
                            plane[:, r0:r1, :],
                        )
    nc.compile()
    return nc


def _host_prep(x, weight, bias):
    # padded, i-tiled x: [B, 128, 2, 66, 68]  (kt inside the free dim so one
    # DMA per row-group covers both contraction halves)
    xp = np.zeros((B, 128, 2, PADH, PADW), dtype=np.float32)
    xp[:, :, :, 1:65, 1:65] = x.reshape(B, 2, 128, H, W).transpose(0, 2, 1, 3, 4)
    xp = np.ascontiguousarray(xp.reshape(B, 128, 2, PADH * PADW))

    # weights as lhsT, phase-blocked:
    # wt[i, kt, phase, j=(a,b), o] = weight[o, kt*128+i, py+2a, px+2b]
    wr = weight.reshape(CO, 2, 128, 4, 4)
    wt = np.empty((128, 2, 4, 4, CO), dtype=np.float32)
    for py in range(2):
        for px in range(2):
            for a in range(2):
                for bb in range(2):
                    wt[:, :, py * 2 + px, a * 2 + bb, :] = (
                        wr[:, :, :, py + 2 * a, px + 2 * bb].transpose(2, 1, 0)
                    )
    wt = np.ascontiguousarray(wt).reshape(128, 2, 16 * CO)

    # bias vectors [128, 4, 9]: interior sum + 4 edge + 4 corner corrections
    bv = np.zeros((128, 4, 9), dtype=np.float32)
    bias = bias.astype(np.float32)
    for py in range(2):
        for px in range(2):
            p = py * 2 + px
            b00 = bias[:, py, px]
            b01 = bias[:, py, px + 2]
            b10 = bias[:, py + 2, px]
            b11 = bias[:, py + 2, px + 2]
            bv[:, p, 0] = b00 + b01 + b10 + b11
            bv[:, p, 1] = -(b10 + b11)   # P=0 row (a=1 invalid)
            bv[:, p, 2] = -(b00 + b01)   # P=64 row (a=0 invalid)
            bv[:, p, 3] = -(b01 + b11)   # Q=0 col (b=1 invalid)
            bv[:, p, 4] = -(b00 + b10)   # Q=64 col (b=0 invalid)
            bv[:, p, 5] = b11            # corner (0,0)
            bv[:, p, 6] = b10            # corner (0,64)
            bv[:, p, 7] = b01            # corner (64,0)
            bv[:, p, 8] = b00            # corner (64,64)
    bv = bv.reshape(128, 36)
    return xp, wt, bv


_NC_CACHE = {}


def _get_nc():
    if "nc" not in _NC_CACHE:
        _NC_CACHE["nc"] = _build_nc()
    return _NC_CACHE["nc"]


def kernel(x, weight, bias, _trace=False, _trace_kwargs=None):
    from concourse.bass_utils import run_bass_kernel_spmd

    x = np.asarray(x, dtype=np.float32)
    weight = np.asarray(weight, dtype=np.float32)
    bias = np.asarray(bias, dtype=np.float32)
    xp, wt, bv = _host_prep(x, weight, bias)

    nc = _get_nc()
    in_maps = [{"xp": xp[b], "wt": wt, "bv": bv} for b in range(B)]
    res = run_bass_kernel_spmd(
        nc, in_maps, list(range(NCORES)),
        trace=_trace, **(_trace_kwargs or {}),
    )
    out = np.empty((B, CO, NH, NW), dtype=np.float32)
    for b in range(B):
        ph = res.results[b]["out"].reshape(4, CO, NP, NP)
        for py in range(2):
            for px in range(2):
                out[b, :, py::2, px::2] = ph[py * 2 + px]
    if _trace:
        kernel._last_results = res
    return out


# revision 24
# speedup vs baseline: 1.3979x; 1.0469x over previous
"""Trainium2 Bass kernel for a stride-2 4x4 ConvTranspose2d with
per-kernel-position bias (bias added before the overlap-add fold).

Shapes (hardcoded):
  x:      (8, 256, 64, 64)  f32
  weight: (128, 256, 4, 4)  f32
  bias:   (128, 4, 4)       f32
  out:    (8, 128, 130, 130) f32   [nh = (64-1)*2+4 = 130]

Strategy: data-parallel over batch — one sample per NeuronCore, 8 cores.
Per core the deconv is computed as 4 output-phase planes (p%2, q%2), each
a 65x65 image. Each phase plane is the sum of 4 shifted matmuls (the
kernel positions sharing that parity) accumulated directly in PSUM:

  plane[o, P, Q] = sum_{a,b in {0,1}} W[:, :, py+2a, px+2b]^T @ xp[:, P-a+1, Q-b+1]

with xp zero-padded so out-of-range taps contribute zero. Matmuls run as
fp32r (full-rate fp32 on the PE array; requires even innermost counts,
hence the 66-wide compute rows of which 65 are kept). The bias is folded
in as a per-partition scalar during the PSUM->SBUF drain, with small
edge/corner corrections for boundary pixels that receive fewer
kernel-position contributions. Phase planes are stored contiguously and
DMA'd out in halves as soon as their rows are final; the host interleaves
the 4 planes into the strided (130,130) output.
"""

import numpy as np

B, CI, H, W = 8, 256, 64, 64
CO, KH, KW = 128, 4, 4
NH = NW = 130
NP = 65          # phase plane side
PADH = 66        # padded x rows
PADW = 68        # padded x cols (fp32r needs even innermost matmul counts;
                 # we compute 66-wide rows and drain the valid 65)
NQ = 66          # matmul output row width (even); col 65 is garbage
NCORES = 8

# P-row chunking of a phase plane: free-dim per chunk must fit one PSUM
# bank (<=512 f32) and stay >=256 for full-rate fp32r.
CHUNK_ROWS = [7, 7, 7, 7, 7, 6, 6, 6, 6, 6]
assert sum(CHUNK_ROWS) == NP
# after chunk index ci, rows [r0, r1) of the plane are complete
OUT_SPLITS = {4: (0, 35), 7: (35, 53), 9: (53, 65)}

XROW_GRPS = [(0, 11), (11, 22), (22, 44), (44, 66)]


def _build_nc():
    import concourse.mybir as mybir
    import concourse.tile as tile
    from concourse import bacc

    f32 = mybir.dt.float32
    f32r = mybir.dt.float32r

    # Bacc (not plain Bass): its compile() runs generate_event_semaphores,
    # legalizing Tile's multi-wait instructions to the 1-wait-per-inst
    # hardware constraint the walrus in this container enforces.
    nc = bacc.Bacc(None)
    xp_ext = nc.declare_dram_parameter("xp", [128, 2, PADH * PADW], f32r, isOutput=False)
    wt_ext = nc.declare_dram_parameter("wt", [128, 2, 16 * CO], f32r, isOutput=False)
    bv_ext = nc.declare_dram_parameter("bv", [128, 4 * 9], f32, isOutput=False)
    out_ext = nc.declare_dram_parameter("out", [4, CO, NP * NP], f32, isOutput=True)

    with tile.TileContext(nc) as tc:
        with (
            tc.tile_pool(name="const", bufs=1) as cpool,
            tc.tile_pool(name="psum", bufs=4, space="PSUM") as ppool,
        ):
            # w_t layout: [i, kt, phase, j=(a,b), o] — phase-blocked so each
            # phase's weights arrive in one small early DMA.
            w_t = cpool.tile([128, 2, 4, 4, CO], f32r, tag="w")
            xp_t = cpool.tile([128, 2, PADH, PADW], f32r, tag="xp")
            bv_t = cpool.tile([128, 4, 9], f32, tag="bv")
            planes = [
                cpool.tile([128, NP, NP], f32, tag=f"plane{p}", name=f"plane{p}")
                for p in range(4)
            ]

            # Inputs split across the two HWDGE rings, first-needed pieces
            # first, so the first matmul is gated by ~1.3MB not the full
            # 6.6MB input load. Late-needed weight phases and the tiny bias
            # vector ride behind on the rings.
            for ph in (0, 1):
                nc.scalar.dma_start(
                    w_t[:, :, ph], wt_ext[:, :, ph * 4 * CO:(ph + 1) * 4 * CO]
                )
            nc.scalar.dma_start(bv_t[:], bv_ext[:])
            for (r0, r1) in XROW_GRPS:
                nc.sync.dma_start(
                    xp_t[:, :, r0:r1, :], xp_ext[:, :, r0 * PADW:r1 * PADW]
                )
            for ph in (2, 3):
                nc.sync.dma_start(
                    w_t[:, :, ph], wt_ext[:, :, ph * 4 * CO:(ph + 1) * 4 * CO]
                )

            # PE warm-up: dummy bf16 matmuls on never-written scratch (no
            # input deps, so they run during the input-load window). HAM
            # un-throttles after ~3.4us of sustained PE activity, so the
            # first real matmul starts at 2.4GHz instead of 1.2.
            warm_in = cpool.tile([128, 512], mybir.dt.bfloat16, tag="warm_in")
            warm_ps = ppool.tile([128, 512], f32, tag="warm_ps")
            nc.vector.memset(warm_in[:], 1.0)
            for _ in range(26):
                nc.tensor.matmul(
                    warm_ps[:], warm_in[:, 0:128], warm_in[:],
                    start=True, stop=True,
                )

            for (py, px) in ((0, 0), (0, 1), (1, 0), (1, 1)):
                pidx = py * 2 + px
                plane = planes[pidx]
                pstart = 0
                for ci, pn in enumerate(CHUNK_ROWS):
                    ps = ppool.tile([128, 7, NQ], f32, tag="acc")
                    mm = 0
                    for a in (0, 1):
                        for b in (0, 1):
                            for kt in (0, 1):
                                lhsT = w_t[:, kt, pidx, a * 2 + b, :]
                                rhs = xp_t[
                                    :, kt,
                                    pstart + 1 - a: pstart + 1 - a + pn,
                                    1 - b: 1 - b + NQ,
                                ]
                                nc.tensor.matmul(
                                    ps[:, :pn, :], lhsT, rhs,
                                    start=(mm == 0), stop=(mm == 7),
                                )
                                mm += 1
                    # drain + interior bias (col 65 of ps is garbage, skip it)
                    nc.vector.tensor_scalar_add(
                        plane[:, pstart:pstart + pn, :],
                        ps[:, :pn, 0:NP],
                        bv_t[:, pidx, 0:1],
                    )
                    pstart += pn

                    if ci in OUT_SPLITS:
                        # rows [r0, r1) are final once their edge corrections
                        # land; stream them out immediately.
                        r0, r1 = OUT_SPLITS[ci]
                        corr = [
                            (plane[:, r0:r1, 0:1], 3),      # Q=0 col
                            (plane[:, r0:r1, 64:65], 4),    # Q=64 col
                        ]
                        if ci == 4:
                            corr = [
                                (plane[:, 0:1, :], 1),      # P=0 row
                                (plane[:, 0:1, 0:1], 5),    # corner (0,0)
                                (plane[:, 0:1, 64:65], 6),  # corner (0,64)
                            ] + corr
                        elif ci == 9:
                            corr = [
                                (plane[:, 64:65, :], 2),      # P=64 row
                                (plane[:, 64:65, 0:1], 7),    # corner (64,0)
                                (plane[:, 64:65, 64:65], 8),  # corner (64,64)
                            ] + corr
                        for (sl, k) in corr:
                            nc.vector.tensor_scalar_add(sl, sl, bv_t[:, pidx, k:k + 1])
                        nc.sync.dma_start(
                            out_ext[pidx, :, r0 * NP:r1 * NP],
                            plane[:, r0:r1, :],
                        )
    nc.compile()
    return nc


def _host_prep(x, weight, bias):
    # padded, i-tiled x: [B, 128, 2, 66, 68]  (kt inside the free dim so one
    # DMA per row-group covers both contraction halves)
    xp = np.zeros((B, 128, 2, PADH, PADW), dtype=np.float32)
    xp[:, :, :, 1:65, 1:65] = x.reshape(B, 2, 128, H, W).transpose(0, 2, 1, 3, 4)
    xp = np.ascontiguousarray(xp.reshape(B, 128, 2, PADH * PADW))

    # weights as lhsT, phase-blocked:
    # wt[i, kt, phase, j=(a,b), o] = weight[o, kt*128+i, py+2a, px+2b]
    wr = weight.reshape(CO, 2, 128, 4, 4)
    wt = np.empty((128, 2, 4, 4, CO), dtype=np.float32)
    for py in range(2):
        for px in range(2):
            for a in range(2):
                for bb in range(2):
                    wt[:, :, py * 2 + px, a * 2 + bb, :] = (
                        wr[:, :, :, py + 2 * a, px + 2 * bb].transpose(2, 1, 0)
                    )
    wt = np.ascontiguousarray(wt).reshape(128, 2, 16 * CO)

    # bias vectors [128, 4, 9]: interior sum + 4 edge + 4 corner corrections
    bv = np.zeros((128, 4, 9), dtype=np.float32)
    bias = bias.astype(np.float32)
    for py in range(2):
        for px in range(2):
            p = py * 2 + px
            b00 = bias[:, py, px]
            b01 = bias[:, py, px + 2]
            b10 = bias[:, py + 2, px]
            b11 = bias[:, py + 2, px + 2]
            bv[:, p, 0] = b00 + b01 + b10 + b11
            bv[:, p, 1] = -(b10 + b11)   # P=0 row (a=1 invalid)
            bv[:, p, 2] = -(b00 + b01)   # P=64 row (a=0 invalid)
            bv[:, p, 3] = -(b01 + b11)   # Q=0 col (b=1 invalid)
            bv[:, p, 4] = -(b00 + b10)   # Q=64 col (b=0 invalid)
            bv[:, p, 5] = b11            # corner (0,0)
            bv[:, p, 6] = b10            # corner (0,64)
            bv[:, p, 7] = b01            # corner (64,0)
            bv[:, p, 8] = b00            # corner (64,64)
    bv = bv.reshape(128, 36)
    return xp, wt, bv


_NC_CACHE = {}


def _get_nc():
    if "nc" not in _NC_CACHE:
        _NC_CACHE["nc"] = _build_nc()
    return _NC_CACHE["nc"]


def kernel(x, weight, bias, _trace=False, _trace_kwargs=None):
    from concourse.bass_utils import run_bass_kernel_spmd

    x = np.asarray(x, dtype=np.float32)
    weight = np.asarray(weight, dtype=np.float32)
    bias = np.asarray(bias, dtype=np.float32)
    xp, wt, bv = _host_prep(x, weight, bias)

    nc = _get_nc()
    in_maps = [{"xp": xp[b], "wt": wt, "bv": bv} for b in range(B)]
    res = run_bass_kernel_spmd(
        nc, in_maps, list(range(NCORES)),
        trace=_trace, **(_trace_kwargs or {}),
    )
    out = np.empty((B, CO, NH, NW), dtype=np.float32)
    for b in range(B):
        ph = res.results[b]["out"].reshape(4, CO, NP, NP)
        for py in range(2):
            for px in range(2):
                out[b, :, py::2, px::2] = ph[py * 2 + px]
    if _trace:
        kernel._last_results = res
    return out
